# revision 11
# baseline (speedup 1.0000x reference)
"""OctreeConv (gather + buffered GEMM) on 8 Trainium2 NeuronCores.

out[n, o] = sum_{k, c} data[neigh[n, k], c] * weights[k, c, o], with
neigh == -1 meaning "no neighbor" (contributes zero).

Strategy (data-parallel over nodes, SPMD on 8 cores):
  - Shard the 200000 nodes into 8 x 25000. Replicate data and weights.
  - -1 indices are remapped on the host to a zero row appended to data,
    so the gather needs no masking and no destination memset.
  - Per 512-node supertile on device:
      108 indirect DMAs (one per (subtile, k); 128 rows of 128B each, one
      row per partition) fill a [128, 4*864] SBUF tile
      (node mod 128 on partitions, (subtile, k, c) along free dim),
      PE transposes 128x128 blocks into [kc, node] PSUM tiles,
      DVE/ACT copy them to SBUF,
      7 accumulating matmuls with W_flat[864, 32] produce out^T [32, 512].
  - Host transposes/concats per-core outputs back to [200000, 32].

Perf notes (measured on HW in this container):
  - Each indirect DMA costs a fixed ~1.39us of serialized Pool/SWDGE time
    (ucode prologue: q7 barrier + index allgather + 128-descriptor gen;
    only Q7 pair 0 generates descriptors). Descriptor payload size and
    HBM locality barely matter (64B vs 128B descs and all-zero indices
    were within 7%), so the kernel is bound by instruction COUNT:
    5292 instructions x ~1.39us ~= 7.3ms per core.
  - The SWDGE ucode consumes exactly one index per partition per
    instruction (multi-index offset APs read only the first index or
    misrender; verified by probes), so 128 rows/instruction is the max.
  - InstDMAGatherAnt would batch ~8k descriptors/instruction at
    0.34ns/desc, but its indices are hard int16 (<=32767-row tables) --
    incompatible with the 200001-row table; all chunked workarounds die
    on either a data-dependent reassembly (scatter wall), 8x PE density
    loss, or 7x drain/gen multiplication.
  - GPSIMD SBUF gathers (ap_gather/indirect_copy) issue one read command
    per <=4 indices with no read overlap on cayman (~25 cyc/idx) -- 25x
    too slow. HWDGE has no indirection support.
  - gpool bufs=8 keeps 8 supertiles of gather DMAs in flight ahead of
    the PE consumers (-3% vs bufs=4).
  - ~9.1% of slots are invalid (-1). A host-side coordinated greedy
    permutes each core's nodes so invalid slots concentrate into whole
    all-invalid (subtile, k) columns -- the SAME (T, t, k) pattern on all
    8 cores (SPMD shares one program; each core picks its own nodes).
    Those ~175 columns/core (3.3%) skip the gather entirely (DVE memset
    instead), saving ~1.4us each (-4.5%). Pad rows are invalid at every
    k and act as universal block fillers. The program is built per skip
    pattern (data-dependent structure; compile is host-side and cached
    per pattern).
"""

import numpy as np

import concourse.bacc as bacc
import concourse.bass as bass
import concourse.mybir as mybir
from concourse.bass_utils import run_bass_kernel_spmd
from concourse.masks import make_identity
from concourse.tile import TileContext

N = 200000
K = 27
C = 32
NCORES = 8
NODES_PER_CORE = N // NCORES  # 25000
SUPER = 512                   # nodes per supertile
SUBT = SUPER // 128           # 4 sub-tiles of 128 nodes
NSUP = (NODES_PER_CORE + SUPER - 1) // SUPER  # 49
NODES_PAD = NSUP * SUPER      # 25088
KC = K * C                    # 864
NBLK = (KC + 127) // 128      # 7 contraction blocks
IDX_W = SUBT * K              # 108 indices per partition per supertile

_PROGRAMS: dict = {}


def _plan_skips(neigh32_full):
    """Coordinated greedy: permute each core's nodes so invalid (-1 -> N)
    slots concentrate into whole (subtile, k) columns, identical across all
    8 cores (SPMD shares one program). Returns per-core node permutations
    (slot -> padded-shard row) and the shared skip list of (T, t, k).

    Each skipped column drops one ~1.39us indirect DMA per supertile pass;
    pad rows (node id == N at every k) act as universal fillers.
    """
    invalid = []
    for j in range(NCORES):
        shard = neigh32_full[j * NODES_PER_CORE : (j + 1) * NODES_PER_CORE]
        inv = shard >= N  # True where no neighbor (remapped to zero row)
        pad = np.ones((NODES_PAD - NODES_PER_CORE, K), dtype=bool)
        invalid.append(np.concatenate([inv, pad], axis=0))  # [25088, 27]

    unplaced = [np.ones(NODES_PAD, dtype=bool) for _ in range(NCORES)]
    perms = [[] for _ in range(NCORES)]
    skips = []  # block index b -> k  (block b occupies slots [128b, 128b+128))
    korder = np.argsort(-sum(inv.sum(axis=0) for inv in invalid))
    for k in korder:
        avail = [np.where(unplaced[j] & invalid[j][:, k])[0] for j in range(NCORES)]
        m = min(len(a) // 128 for a in avail)
        for b in range(m):
            for j in range(NCORES):
                blk = avail[j][b * 128 : (b + 1) * 128]
                perms[j].extend(blk.tolist())
                unplaced[j][blk] = False
            skips.append(int(k))
    for j in range(NCORES):
        rest = np.where(unplaced[j])[0]
        perms[j] = np.asarray(perms[j] + rest.tolist(), dtype=np.int64)

    skip_set = frozenset((b // SUBT, b % SUBT, k) for b, k in enumerate(skips))
    return perms, skip_set


def _build_program(reps: int = 1, skip_set: frozenset = frozenset()) -> bass.Bass:
    nc = bacc.Bacc("TRN2", target_bir_lowering=False, debug=False)
    f32 = mybir.dt.float32

    data = nc.dram_tensor("data", [N + 1, C], f32, kind="ExternalInput")
    wflat = nc.dram_tensor("wflat", [NBLK * 128, C], f32, kind="ExternalInput")
    nidx = nc.dram_tensor(
        "nidx", [128, NSUP * IDX_W], mybir.dt.int32, kind="ExternalInput"
    )
    out = nc.dram_tensor("out", [NSUP, C, SUPER], f32, kind="ExternalOutput")

    with TileContext(nc) as tc:
        with (
            tc.tile_pool(name="const", bufs=1) as cpool,
            tc.tile_pool(name="gpool", bufs=8) as gpool,
            tc.tile_pool(name="gtpool", bufs=4) as gtpool,
            tc.tile_pool(name="opool", bufs=3) as opool,
            tc.tile_pool(name="pst", bufs=2, space="PSUM") as pst,
            tc.tile_pool(name="pso", bufs=2, space="PSUM") as pso,
        ):
            ident = cpool.tile([128, 128], f32)
            make_identity(nc, ident)

            # w_sb[p, b, c] = wflat[b*128 + p, c]; one DMA for all blocks
            w_sb = cpool.tile([128, NBLK, C], f32)
            nc.sync.dma_start(
                out=w_sb[:],
                in_=wflat.rearrange("(b p) c -> p b c", p=128),
            )

            nidx_sb = cpool.tile([128, NSUP * IDX_W], mybir.dt.int32)
            nc.sync.dma_start(out=nidx_sb[:], in_=nidx[:])

            # PE warmup: observe the ident (Pool) and w_sb (DMA) semaphores
            # before the main loop. The hardware LDWEIGHTS slot only carries
            # a single sync wait, so each steady-state matmul may wait on at
            # most one semaphore.
            warm_ps = pst.tile([128, SUPER], f32)
            nc.tensor.transpose(
                out=warm_ps[:128, :128], in_=ident[:], identity=ident[:]
            )
            warm_po = pso.tile([C, SUPER], f32)
            nc.tensor.matmul(
                out=warm_po[:, :128],
                lhsT=w_sb[:, 0, :],
                rhs=ident[:],
                start=True,
                stop=True,
            )

            for T in [t for _ in range(reps) for t in range(NSUP)]:
                g = gpool.tile([128, SUBT * KC], f32)
                # One indirect DMA moves 128 rows (one index per partition):
                # gather slot (t, k) of this supertile per instruction.
                # Columns known to be all-invalid are memset on DVE instead
                # (saves the ~1.39us Pool/SWDGE fixed cost per column).
                for t in range(SUBT):
                    for k in range(K):
                        sl = g[:, (t * K + k) * C : (t * K + k + 1) * C]
                        if (T, t, k) in skip_set:
                            nc.vector.memset(sl, 0.0)
                            continue
                        col = T * IDX_W + t * K + k
                        nc.gpsimd.indirect_dma_start(
                            out=sl,
                            out_offset=None,
                            in_=data[:],
                            in_offset=bass.IndirectOffsetOnAxis(
                                ap=nidx_sb[:, col : col + 1],
                                axis=0,
                            ),
                        )

                opsum = pso.tile([C, SUPER], f32)
                for b in range(NBLK):
                    rows = min(128, KC - 128 * b)
                    gt_ps = pst.tile([128, SUPER], f32)
                    for t in range(SUBT):
                        nc.tensor.transpose(
                            out=gt_ps[:rows, t * 128 : (t + 1) * 128],
                            in_=g[:, t * KC + 128 * b : t * KC + 128 * b + rows],
                            identity=ident[:],
                        )
                    gt_sb = gtpool.tile([128, SUPER], f32)
                    if b % 2 == 0:
                        nc.vector.tensor_copy(out=gt_sb[:rows, :], in_=gt_ps[:rows, :])
                    else:
                        nc.scalar.copy(out=gt_sb[:rows, :], in_=gt_ps[:rows, :])
                    nc.tensor.matmul(
                        out=opsum[:],
                        lhsT=w_sb[:rows, b, :],
                        rhs=gt_sb[:rows, :],
                        start=(b == 0),
                        stop=(b == NBLK - 1),
                    )

                o_sb = opool.tile([C, SUPER], f32)
                nc.scalar.copy(out=o_sb[:], in_=opsum[:])
                nc.sync.dma_start(out=out[T], in_=o_sb[:])

    nc.compile()
    return nc


def _get_program(skip_set: frozenset) -> bass.Bass:
    if skip_set not in _PROGRAMS:
        _PROGRAMS[skip_set] = _build_program(skip_set=skip_set)
    return _PROGRAMS[skip_set]


def _prep_core_inputs(data_pad, wflat, neigh32, perms=None):
    """Build the 8 per-core input maps from full inputs. `perms` (from
    _plan_skips) reorders each core's padded shard rows; slot i of the
    device layout holds padded-shard row perms[j][i]."""
    in_maps = []
    for j in range(NCORES):
        shard = neigh32[j * NODES_PER_CORE : (j + 1) * NODES_PER_CORE]
        pad = np.full((NODES_PAD - NODES_PER_CORE, K), N, dtype=np.int32)
        shard = np.concatenate([shard, pad], axis=0)  # [25088, 27]
        if perms is not None:
            shard = shard[perms[j]]
        # nidx[p, T*IDX_W + t*K + k] = shard[(T*SUBT + t)*128 + p, k]
        nidx = (
            shard.reshape(NSUP, SUBT, 128, K)
            .transpose(2, 0, 1, 3)
            .reshape(128, NSUP * IDX_W)
        )
        in_maps.append(
            {
                "data": data_pad,
                "wflat": wflat,
                "nidx": np.ascontiguousarray(nidx),
            }
        )
    return in_maps


def kernel(data, weights, neigh):
    data = np.asarray(data, dtype=np.float32)
    weights = np.asarray(weights, dtype=np.float32)
    neigh = np.asarray(neigh)

    data_pad = np.zeros((N + 1, C), dtype=np.float32)
    data_pad[:N] = data
    wflat = np.zeros((NBLK * 128, C), dtype=np.float32)
    wflat[:KC] = weights.reshape(KC, C)
    neigh32 = neigh.astype(np.int32)
    neigh32[neigh32 < 0] = N  # zero row

    perms, skip_set = _plan_skips(neigh32)
    nc = _get_program(skip_set)
    in_maps = _prep_core_inputs(data_pad, wflat, neigh32, perms)
    res = run_bass_kernel_spmd(nc, in_maps, core_ids=list(range(NCORES)))

    outs = []
    for j in range(NCORES):
        o = np.asarray(res.results[j]["out"])  # [NSUP, C, SUPER]
        o = o.transpose(0, 2, 1).reshape(NODES_PAD, C)
        unperm = np.empty_like(o)
        unperm[perms[j]] = o  # slot i holds padded-shard row perms[j][i]
        outs.append(unperm[:NODES_PER_CORE])
    return np.ascontiguousarray(np.concatenate(outs, axis=0), dtype=np.float32)



# revision 13
# speedup vs baseline: 1.0198x; 1.0198x over previous
"""OctreeConv (gather + buffered GEMM) on 8 Trainium2 NeuronCores.

out[n, o] = sum_{k, c} data[neigh[n, k], c] * weights[k, c, o], with
neigh == -1 meaning "no neighbor" (contributes zero).

Strategy (data-parallel over nodes, SPMD on 8 cores):
  - Shard the 200000 nodes into 8 x 25000. Replicate data and weights.
  - -1 indices are remapped on the host to a zero row appended to data,
    so the gather needs no masking and no destination memset.
  - Per 512-node supertile on device:
      108 indirect DMAs (one per (subtile, k); 128 rows of 128B each, one
      row per partition) fill a [128, 4*864] SBUF tile
      (node mod 128 on partitions, (subtile, k, c) along free dim),
      PE transposes 128x128 blocks into [kc, node] PSUM tiles,
      DVE/ACT copy them to SBUF,
      7 accumulating matmuls with W_flat[864, 32] produce out^T [32, 512].
  - Host transposes/concats per-core outputs back to [200000, 32].

Perf notes (measured on HW in this container):
  - Each indirect DMA costs a fixed ~1.39us of serialized Pool/SWDGE time
    (ucode prologue: q7 barrier + index allgather + 128-descriptor gen;
    only Q7 pair 0 generates descriptors). Descriptor payload size and
    HBM locality barely matter (64B vs 128B descs and all-zero indices
    were within 7%), so the kernel is bound by instruction COUNT:
    5292 instructions x ~1.39us ~= 7.3ms per core.
  - The SWDGE ucode consumes exactly one index per partition per
    instruction (multi-index offset APs read only the first index or
    misrender; verified by probes), so 128 rows/instruction is the max.
  - InstDMAGatherAnt would batch ~8k descriptors/instruction at
    0.34ns/desc, but its indices are hard int16 (<=32767-row tables) --
    incompatible with the 200001-row table; all chunked workarounds die
    on either a data-dependent reassembly (scatter wall), 8x PE density
    loss, or 7x drain/gen multiplication.
  - GPSIMD SBUF gathers (ap_gather/indirect_copy) issue one read command
    per <=4 indices with no read overlap on cayman (~25 cyc/idx) -- 25x
    too slow. HWDGE has no indirection support.
  - gpool bufs=8 keeps 8 supertiles of gather DMAs in flight ahead of
    the PE consumers (-3% vs bufs=4).
  - ~9.1% of slots are invalid (-1). A host-side coordinated greedy
    permutes each core's nodes so invalid slots concentrate into whole
    all-invalid (subtile, k) columns -- the SAME (T, t, k) pattern on all
    8 cores (SPMD shares one program; each core picks its own nodes).
    Those ~175 columns/core (3.3%) skip the gather entirely (DVE memset
    instead), saving ~1.4us each (-4.5%). Pad rows are invalid at every
    k and act as universal block fillers. The program is built per skip
    pattern (data-dependent structure; compile is host-side and cached
    per pattern).
"""

import numpy as np

import concourse.bacc as bacc
import concourse.bass as bass
import concourse.mybir as mybir
from concourse.bass_utils import run_bass_kernel_spmd
from concourse.masks import make_identity
from concourse.tile import TileContext

N = 200000
K = 27
C = 32
NCORES = 8
NODES_PER_CORE = N // NCORES  # 25000
SUPER = 512                   # nodes per supertile
SUBT = SUPER // 128           # 4 sub-tiles of 128 nodes
NSUP = (NODES_PER_CORE + SUPER - 1) // SUPER  # 49
NODES_PAD = NSUP * SUPER      # 25088
KC = K * C                    # 864
NBLK = (KC + 127) // 128      # 7 contraction blocks
IDX_W = SUBT * K              # 108 indices per partition per supertile

_PROGRAMS: dict = {}


def _plan_skips(neigh32_full):
    """Coordinated greedy: permute each core's nodes so invalid (-1 -> N)
    slots concentrate into whole (subtile, k) columns, identical across all
    8 cores (SPMD shares one program). Returns per-core node permutations
    (slot -> padded-shard row) and the shared skip list of (T, t, k).

    Each skipped column drops one ~1.39us indirect DMA per supertile pass;
    pad rows (node id == N at every k) act as universal fillers.
    """
    invalid = []
    for j in range(NCORES):
        shard = neigh32_full[j * NODES_PER_CORE : (j + 1) * NODES_PER_CORE]
        inv = shard >= N  # True where no neighbor (remapped to zero row)
        pad = np.ones((NODES_PAD - NODES_PER_CORE, K), dtype=bool)
        invalid.append(np.concatenate([inv, pad], axis=0))  # [25088, 27]

    MAXBLK = NSUP * SUBT
    unplaced = [np.ones(NODES_PAD, dtype=bool) for _ in range(NCORES)]
    perms = [[] for _ in range(NCORES)]
    blocks = []  # block b -> tuple of skipped k's (block b = subtile b)

    def take_block(masks):
        """Place one 128-node block per core from the given candidate masks,
        preferring nodes with the fewest invalid k's (save flexible ones)."""
        for j in range(NCORES):
            cand = np.where(masks[j])[0]
            order = np.argsort(invalid[j][cand].sum(axis=1), kind="stable")
            blk = cand[order[:128]]
            perms[j].extend(blk.tolist())
            unplaced[j][blk] = False

    # Phase 1: pair-pure blocks (all 128 nodes invalid at BOTH k1 and k2)
    # skip two instructions per block (64 nodes per skip vs 128).
    kpairs = [(a, b) for a in range(K) for b in range(a + 1, K)]
    while len(blocks) < MAXBLK:
        cnt = np.empty((NCORES, len(kpairs)), dtype=np.int64)
        for j in range(NCORES):
            iv = invalid[j][unplaced[j]].astype(np.int32)
            gram = iv.T @ iv  # [K, K] co-invalid counts
            cnt[j] = [gram[a, b] for a, b in kpairs]
        mins = cnt.min(axis=0)
        pi = int(mins.argmax())
        if mins[pi] < 128:
            break
        a, b = kpairs[pi]
        take_block(
            [unplaced[j] & invalid[j][:, a] & invalid[j][:, b] for j in range(NCORES)]
        )
        blocks.append((a, b))

    # Phase 2: single-k blocks from the remaining pool.
    while len(blocks) < MAXBLK:
        cnt = np.empty((NCORES, K), dtype=np.int64)
        for j in range(NCORES):
            iv = invalid[j][unplaced[j]]
            cnt[j] = iv.sum(axis=0)
        mins = cnt.min(axis=0)
        k = int(mins.argmax())
        if mins[k] < 128:
            break
        take_block([unplaced[j] & invalid[j][:, k] for j in range(NCORES)])
        blocks.append((k,))

    for j in range(NCORES):
        rest = np.where(unplaced[j])[0]
        perms[j] = np.asarray(perms[j] + rest.tolist(), dtype=np.int64)

    skip_set = frozenset(
        (b // SUBT, b % SUBT, k) for b, ks in enumerate(blocks) for k in ks
    )
    return perms, skip_set


def _build_program(reps: int = 1, skip_set: frozenset = frozenset()) -> bass.Bass:
    nc = bacc.Bacc("TRN2", target_bir_lowering=False, debug=False)
    f32 = mybir.dt.float32

    data = nc.dram_tensor("data", [N + 1, C], f32, kind="ExternalInput")
    wflat = nc.dram_tensor("wflat", [NBLK * 128, C], f32, kind="ExternalInput")
    nidx = nc.dram_tensor(
        "nidx", [128, NSUP * IDX_W], mybir.dt.int32, kind="ExternalInput"
    )
    out = nc.dram_tensor("out", [NSUP, C, SUPER], f32, kind="ExternalOutput")

    with TileContext(nc) as tc:
        with (
            tc.tile_pool(name="const", bufs=1) as cpool,
            tc.tile_pool(name="gpool", bufs=8) as gpool,
            tc.tile_pool(name="gtpool", bufs=4) as gtpool,
            tc.tile_pool(name="opool", bufs=3) as opool,
            tc.tile_pool(name="pst", bufs=2, space="PSUM") as pst,
            tc.tile_pool(name="pso", bufs=2, space="PSUM") as pso,
        ):
            ident = cpool.tile([128, 128], f32)
            make_identity(nc, ident)

            # w_sb[p, b, c] = wflat[b*128 + p, c]; one DMA for all blocks
            w_sb = cpool.tile([128, NBLK, C], f32)
            nc.sync.dma_start(
                out=w_sb[:],
                in_=wflat.rearrange("(b p) c -> p b c", p=128),
            )

            nidx_sb = cpool.tile([128, NSUP * IDX_W], mybir.dt.int32)
            nc.sync.dma_start(out=nidx_sb[:], in_=nidx[:])

            # PE warmup: observe the ident (Pool) and w_sb (DMA) semaphores
            # before the main loop. The hardware LDWEIGHTS slot only carries
            # a single sync wait, so each steady-state matmul may wait on at
            # most one semaphore.
            warm_ps = pst.tile([128, SUPER], f32)
            nc.tensor.transpose(
                out=warm_ps[:128, :128], in_=ident[:], identity=ident[:]
            )
            warm_po = pso.tile([C, SUPER], f32)
            nc.tensor.matmul(
                out=warm_po[:, :128],
                lhsT=w_sb[:, 0, :],
                rhs=ident[:],
                start=True,
                stop=True,
            )

            for T in [t for _ in range(reps) for t in range(NSUP)]:
                g = gpool.tile([128, SUBT * KC], f32)
                # One indirect DMA moves 128 rows (one index per partition):
                # gather slot (t, k) of this supertile per instruction.
                # Columns known to be all-invalid are memset on DVE instead
                # (saves the ~1.39us Pool/SWDGE fixed cost per column).
                for t in range(SUBT):
                    for k in range(K):
                        sl = g[:, (t * K + k) * C : (t * K + k + 1) * C]
                        if (T, t, k) in skip_set:
                            nc.vector.memset(sl, 0.0)
                            continue
                        col = T * IDX_W + t * K + k
                        nc.gpsimd.indirect_dma_start(
                            out=sl,
                            out_offset=None,
                            in_=data[:],
                            in_offset=bass.IndirectOffsetOnAxis(
                                ap=nidx_sb[:, col : col + 1],
                                axis=0,
                            ),
                        )

                opsum = pso.tile([C, SUPER], f32)
                for b in range(NBLK):
                    rows = min(128, KC - 128 * b)
                    gt_ps = pst.tile([128, SUPER], f32)
                    for t in range(SUBT):
                        nc.tensor.transpose(
                            out=gt_ps[:rows, t * 128 : (t + 1) * 128],
                            in_=g[:, t * KC + 128 * b : t * KC + 128 * b + rows],
                            identity=ident[:],
                        )
                    gt_sb = gtpool.tile([128, SUPER], f32)
                    if b % 2 == 0:
                        nc.vector.tensor_copy(out=gt_sb[:rows, :], in_=gt_ps[:rows, :])
                    else:
                        nc.scalar.copy(out=gt_sb[:rows, :], in_=gt_ps[:rows, :])
                    nc.tensor.matmul(
                        out=opsum[:],
                        lhsT=w_sb[:rows, b, :],
                        rhs=gt_sb[:rows, :],
                        start=(b == 0),
                        stop=(b == NBLK - 1),
                    )

                o_sb = opool.tile([C, SUPER], f32)
                nc.scalar.copy(out=o_sb[:], in_=opsum[:])
                nc.sync.dma_start(out=out[T], in_=o_sb[:])

    nc.compile()
    return nc


def _get_program(skip_set: frozenset) -> bass.Bass:
    if skip_set not in _PROGRAMS:
        _PROGRAMS[skip_set] = _build_program(skip_set=skip_set)
    return _PROGRAMS[skip_set]


def _prep_core_inputs(data_pad, wflat, neigh32, perms=None):
    """Build the 8 per-core input maps from full inputs. `perms` (from
    _plan_skips) reorders each core's padded shard rows; slot i of the
    device layout holds padded-shard row perms[j][i]."""
    in_maps = []
    for j in range(NCORES):
        shard = neigh32[j * NODES_PER_CORE : (j + 1) * NODES_PER_CORE]
        pad = np.full((NODES_PAD - NODES_PER_CORE, K), N, dtype=np.int32)
        shard = np.concatenate([shard, pad], axis=0)  # [25088, 27]
        if perms is not None:
            shard = shard[perms[j]]
        # nidx[p, T*IDX_W + t*K + k] = shard[(T*SUBT + t)*128 + p, k]
        nidx = (
            shard.reshape(NSUP, SUBT, 128, K)
            .transpose(2, 0, 1, 3)
            .reshape(128, NSUP * IDX_W)
        )
        in_maps.append(
            {
                "data": data_pad,
                "wflat": wflat,
                "nidx": np.ascontiguousarray(nidx),
            }
        )
    return in_maps


def kernel(data, weights, neigh):
    data = np.asarray(data, dtype=np.float32)
    weights = np.asarray(weights, dtype=np.float32)
    neigh = np.asarray(neigh)

    data_pad = np.zeros((N + 1, C), dtype=np.float32)
    data_pad[:N] = data
    wflat = np.zeros((NBLK * 128, C), dtype=np.float32)
    wflat[:KC] = weights.reshape(KC, C)
    neigh32 = neigh.astype(np.int32)
    neigh32[neigh32 < 0] = N  # zero row

    perms, skip_set = _plan_skips(neigh32)
    nc = _get_program(skip_set)
    in_maps = _prep_core_inputs(data_pad, wflat, neigh32, perms)
    res = run_bass_kernel_spmd(nc, in_maps, core_ids=list(range(NCORES)))

    outs = []
    for j in range(NCORES):
        o = np.asarray(res.results[j]["out"])  # [NSUP, C, SUPER]
        o = o.transpose(0, 2, 1).reshape(NODES_PAD, C)
        unperm = np.empty_like(o)
        unperm[perms[j]] = o  # slot i holds padded-shard row perms[j][i]
        outs.append(unperm[:NODES_PER_CORE])
    return np.ascontiguousarray(np.concatenate(outs, axis=0), dtype=np.float32)

